# revision 13
# baseline (speedup 1.0000x reference)
"""Trainium2 Bass kernel for nn_CriticNetwork (gnn_message_passing).

Math (verified vs reference): the reference broadcasts edge_index to
(B, 2, E) and reshapes to (2, B*E); row-major interleaving makes src and
dst elementwise equal, so every edge is a self-edge and both GCNConv
layers collapse to plain linear layers (deg*x/deg = x):

    x = relu(x @ W1); x = relu(x @ W2)        (biases are zeros)
    node_avg[b] = mean_n(x[b,n] @ node_fc_W) + node_fc_b
    col path is a tiny 2-layer MLP; final head is a [4,2] MLP.

Device strategy (per core, 25000 nodes = half a batch):
  * node path in fp8e4m3 end to end (x, W1*16, W2*16, hidden acts);
    ~2e-3 final rel err vs the 2e-2 gate.  The tiny col path stays bf16
    (quantizing it dominates the error budget).
  * L1 uses MatmulPerfMode.DoubleRow: 256-deep contraction packs FOUR
    nodes per output column.  Outputs must start at PSUM partition 0,
    so the two 4-node bands go to separate [64,512] banks; relu1 band A
    runs on ACT and band B on DVE, writing halves of one dense
    [128,512] h1r tile; L2 is one plain fp8 matmul per chunk; relu2 +
    row-sum accum alternates ACT/DVE by chunk parity.
  * ALL inputs ship in ONE fp8 DRAM tensor (weights fp8 | bf16 block as
    raw bytes via AP.bitcast | x blocks), moved by 5 column-range DMAs
    round-robined over the two HWDGE queues (Sync + Scalar) so the
    hardware descriptor generators work in parallel; x stays fully
    resident in SBUF (no pool recycling, DMA never stalls on compute).
  * the final reduction is done ON the PE: stats row-sums (bf16) dot
    node_fc_W/(256N) and col_W2/C, so the output DMA is a single [1,2]
    fp32 descriptor.  (A [128,1] output DMA costs ~128 descriptors and
    microseconds of tail latency.)
"""

import ml_dtypes
import numpy as np

import concourse.bacc as bacc
import concourse.bass as bass
import concourse.mybir as mybir
import concourse.tile as tile
from concourse.bass_utils import run_bass_kernel_spmd

P = 128
N_CORES = 8
B, N, F_NODE, H = 4, 50000, 64, 16
C, F_COL = 1000, 32
NODES = (B * N) // N_CORES                 # 25000 nodes per core
CHN = 4096                                 # nodes per PSUM chunk (8/col x 512)
NQ_FULL = NODES // CHN                     # 6 full chunks
TAIL_N = NODES - NQ_FULL * CHN             # 424
TAIL_U = TAIL_N // 8                       # 53 output columns in tail chunk
NCH = NQ_FULL + 1                          # 7 chunks total
COLN = (B * C) // N_CORES                  # 500 col-feature rows per core
WS = 16.0                                  # pow2 weight scale for fp8

F8 = mybir.dt.float8e4
NPF8 = ml_dtypes.float8_e4m3               # matches mybir.dt.np(float8e4)
BF = mybir.dt.bfloat16
NPBF = ml_dtypes.bfloat16

# Single packed input tensor xw8 [128, TOT] fp8 (per core), columns:
#   [0:128)        wl1: W1 DoubleRow blockdiag ([128,2,64] view)
#   [128:256)      wl2: blockdiag(W2 x8) flattened
#   [256:257)      zeros column (warmup operand)
#   [320:...)      bf16 block as raw bytes (bitcast view [128, WBX]):
#                  bf16 cols 0:16 col_W1 (rows 0-31), 16:516 colT
#                  (rows 0-31), 516 wn, 517 wc, 518 b1s, 519 b2s, 520 cb1
#   [X0:X0+12288)  x main blocks [24, 512]
#   [XT:XT+212)    x tail blocks [4, 53]
WBX = H + COLN + 5                          # 521 bf16 columns
BF_OFF = 320
X0 = BF_OFF + 2 * WBX                       # 1362 -> pad to 1364
X0 = (X0 + 3) & ~3                          # 1364
XT = X0 + NQ_FULL * 2048                    # 13652
TOT = XT + 4 * TAIL_U                       # 13864

PROFILE = False
CHECK_WAITS = True
LAST_EXEC_TIME_NS = None
LAST_RESULTS = None

_NC_CACHE = {}


def _build_nc(with_bias=False):
    f32 = mybir.dt.float32
    Relu = mybir.ActivationFunctionType.Relu
    DR = mybir.MatmulPerfMode.DoubleRow
    nc = bacc.Bacc("TRN2")

    xw8 = nc.dram_tensor("xw8", [P, TOT], F8, kind="ExternalInput")
    out = nc.dram_tensor("out", [1, 2], f32, kind="ExternalOutput")

    with tile.TileContext(nc) as tc:
        with (
            tc.tile_pool(name="consts", bufs=1) as consts,
            tc.tile_pool(name="xin", bufs=1) as xin,
            tc.tile_pool(name="work", bufs=1) as work,
            tc.tile_pool(name="psum", bufs=1, space="PSUM") as psum,
        ):
            xf = xin.tile([P, TOT], F8)
            stats = consts.tile([P, 5], f32)
            node_sb = consts.tile([P, 1], BF)
            col_sb = consts.tile([H, 1], f32)
            col_sbb = consts.tile([H, 1], BF)
            outsb = consts.tile([1, 2], f32)
            colscr = consts.tile([H, COLN], BF)
            h1r = [work.tile([P, 1024], F8, tag=f"h1r{k}", name=f"h1r{k}")
                   for k in range(2)]
            h1rt = work.tile([P, 64], F8, tag="h1rt", name="h1rt")
            scr = [work.tile([P, 1024], F8, tag=f"scr{k}", name=f"scr{k}")
                   for k in range(2)]

            # DoubleRow matmul outputs must start at PSUM partition 0, so
            # the two 4-node bands land in separate half-used banks.
            # Chunks are processed in PAIRS: each relu op covers [.,1024]
            # spanning two adjacent banks, halving per-op overhead.
            ps1a = psum.tile([64, 1024], f32, tag="ps1a", name="ps1a")
            ps1b = psum.tile([64, 1024], f32, tag="ps1b", name="ps1b")
            ps2 = psum.tile([P, 1024], f32, tag="ps2", name="ps2")
            ps2c = psum.tile([P, 512], f32, tag="ps2c", name="ps2c")
            pscm = psum.tile([H, 512], f32, tag="pscm")
            psc = pscm[:, 0:COLN]
            psd = pscm[0:1, COLN:COLN + 1]
            psdot = pscm[0:1, COLN + 1:COLN + 3]

            wl1 = xf[:, 0:128].rearrange("p (t m) -> p t m", t=2)
            wl2 = xf[:, 128:256]
            zc = xf[:, 256:257]
            wbx = xf[:, BF_OFF:BF_OFF + 2 * WBX].bitcast(BF)
            cw1 = wbx[0:F_COL, 0:H]
            colT = wbx[0:F_COL, H:H + COLN]
            wn = wbx[:, H + COLN:H + COLN + 1]
            wc = wbx[0:H, H + COLN + 1:H + COLN + 2]
            b1s = wbx[:, H + COLN + 2:H + COLN + 3]
            b2s = wbx[:, H + COLN + 3:H + COLN + 4]
            cb1 = wbx[0:H, H + COLN + 4:H + COLN + 5]

            # stats zeroed on GpSimd: no data deps, runs during DMA window.
            nc.gpsimd.memset(stats[:], 0.0)

            # DMA plan: consts ride the third (GpSimd SWDGE) queue; the
            # first chunk pair is split by partition halves across both
            # HWDGE queues so their descriptor generators start in
            # parallel; remaining x goes in two big pieces.
            nc.sync.dma_start(xf[0:64, X0:X0 + 4096], xw8[0:64, X0:X0 + 4096])
            nc.scalar.dma_start(xf[64:128, X0:X0 + 4096],
                                xw8[64:128, X0:X0 + 4096])
            nc.gpsimd.dma_start(xf[:, 0:X0], xw8[:, 0:X0])
            nc.sync.dma_start(xf[:, X0 + 4096:X0 + 8192],
                              xw8[:, X0 + 4096:X0 + 8192])
            nc.scalar.dma_start(xf[:, X0 + 8192:TOT], xw8[:, X0 + 8192:TOT])

            # Warmup matmul: subsumes the consts-piece DMA wait into PE
            # program order with a single semaphore wait.
            nc.tensor.matmul(psd, zc, zc, start=True, stop=True)
            nc.vector.tensor_copy(stats[0:1, 4:5], psd)

            def relu1(eng, dst, src, u):
                if with_bias:
                    if eng is nc.scalar:
                        nc.scalar.activation(dst, src, Relu, bias=b1s[0:64])
                    else:
                        # b1s is 16-periodic; rows 0:64 serve band B too
                        nc.vector.tensor_scalar(
                            dst, src, b1s[0:64], 0.0,
                            mybir.AluOpType.add, mybir.AluOpType.max)
                elif eng is nc.scalar:
                    nc.scalar.activation(dst, src, Relu)
                else:
                    nc.vector.tensor_scalar_max(dst, src, 0.0)

            def relu2(eng, dst, src, acc):
                if with_bias:
                    if eng is nc.scalar:
                        nc.scalar.activation(dst, src, Relu, bias=b2s,
                                             accum_out=acc)
                    else:
                        nc.vector.tensor_scalar(
                            dst, src, b2s, 0.0,
                            mybir.AluOpType.add, mybir.AluOpType.max,
                            accum_out=acc)
                elif eng is nc.scalar:
                    nc.scalar.activation(dst, src, Relu, accum_out=acc)
                else:
                    nc.vector.tensor_scalar(
                        dst, src, 0.0, 0.0,
                        mybir.AluOpType.max, mybir.AluOpType.add,
                        accum_out=acc)

            # 3 chunk pairs + tail.  relu1A on ACT, relu1B on DVE; relu2
            # pairs split ACT/ACT/DVE + tail on DVE to balance engines.
            r2eng = [nc.scalar, nc.vector, nc.scalar]
            for pr in range(3):
                h = h1r[pr % 2]
                for j in range(2):
                    base = X0 + 2048 * (2 * pr + j)
                    xa = xf[:, base:base + 1024].rearrange(
                        "p (t u) -> p t u", t=2)
                    xb = xf[:, base + 1024:base + 2048].rearrange(
                        "p (t u) -> p t u", t=2)
                    nc.tensor.matmul(ps1a[:, 512 * j:512 * j + 512], wl1, xa,
                                     start=True, stop=True, perf_mode=DR,
                                     tile_position=(0, 0))
                    nc.tensor.matmul(ps1b[:, 512 * j:512 * j + 512], wl1, xb,
                                     start=True, stop=True, perf_mode=DR,
                                     tile_position=(0, 0))
                relu1(nc.scalar, h[0:64, :], ps1a[:, :], 1024)
                relu1(nc.vector, h[64:128, :], ps1b[:, :], 1024)
                for j in range(2):
                    nc.tensor.matmul(ps2[:, 512 * j:512 * j + 512], wl2,
                                     h[:, 512 * j:512 * j + 512],
                                     start=True, stop=True)
                relu2(r2eng[pr], scr[pr % 2][:, :], ps2[:, :],
                      stats[:, pr:pr + 1])
                if pr == 0:
                    # col path (bf16, tiny): fills PE/ACT slack early
                    nc.tensor.matmul(psc[:, :], cw1, colT, start=True,
                                     stop=True)
                    nc.scalar.activation(colscr[:], psc[:], Relu,
                                         bias=cb1 if with_bias else 0.0,
                                         accum_out=col_sb[:])
                    nc.vector.tensor_copy(col_sbb[:], col_sb[:])

            # tail chunk (53 columns)
            u = TAIL_U
            base = XT
            xa = xf[:, base:base + 2 * u].rearrange("p (t u) -> p t u", t=2)
            xb = xf[:, base + 2 * u:base + 4 * u].rearrange(
                "p (t u) -> p t u", t=2)
            nc.tensor.matmul(ps1a[:, :u], wl1, xa, start=True, stop=True,
                             perf_mode=DR, tile_position=(0, 0))
            nc.tensor.matmul(ps1b[:, :u], wl1, xb, start=True, stop=True,
                             perf_mode=DR, tile_position=(0, 0))
            relu1(nc.scalar, h1rt[0:64, :u], ps1a[:, :u], u)
            relu1(nc.vector, h1rt[64:128, :u], ps1b[:, :u], u)
            nc.tensor.matmul(ps2c[:, :u], wl2, h1rt[:, :u],
                             start=True, stop=True)
            relu2(nc.vector, scr[0][:, :u], ps2c[:, :u], stats[:, 3:4])

            # Final reductions on-chip: bf16 row sums, then PE dot products
            # so the output is a single tiny [1,2] DMA (one descriptor).
            with nc.allow_low_precision(
                    reason="bf16 row-sums feed a bf16 PE dot; 8 fp32 "
                           "terms/partition, ~0.4% quantization on a "
                           "2e-2 budget"):
                nc.vector.tensor_reduce(node_sb[:], stats[:],
                                        axis=mybir.AxisListType.X,
                                        op=mybir.AluOpType.add)
            nc.tensor.matmul(psdot[0:1, 0:1], node_sb[:], wn,
                             start=True, stop=True)
            nc.tensor.matmul(psdot[0:1, 1:2], col_sbb[:], wc,
                             start=True, stop=True)
            nc.scalar.copy(outsb[:], psdot[:])
            nc.sync.dma_start(out[:], outsb[:])

    nc.finalize()

    if CHECK_WAITS:
        for blk in nc.m.functions[0].blocks:
            for inst in blk.instructions:
                si = inst.sync_info
                nwait = len(si.on_wait) if si and si.on_wait else 0
                limit = 2 if type(inst).__name__ in (
                    "InstEventSemaphore", "InstDrain", "InstDMACopy") else 1
                assert nwait <= limit, (
                    inst.name, type(inst).__name__,
                    [w.ant_name for w in si.on_wait])
    return nc


def _get_nc(with_bias=False):
    key = ("nc", with_bias)
    if key not in _NC_CACHE:
        _NC_CACHE[key] = _build_nc(with_bias)
    return _NC_CACHE[key]


def _pack_x(node_features):
    """[B,N,64] -> per-core [128, 12288] + [128, 212] fp8 x payloads.

    Node n (within a core) lives at chunk q = n // 4096, slot
    s = (n % 4096) // 512 = band*4 + t*2 + half, column u = n % 512:
    columns q*2048 + band*1024 + t*512 + u, partitions half*64 + f.
    """
    x = np.ascontiguousarray(node_features, np.float32).reshape(
        N_CORES, NODES, F_NODE)
    main = x[:, :NQ_FULL * CHN].reshape(N_CORES, NQ_FULL, 2, 2, 2, 512, F_NODE)
    #                  core, q, band, t, half, u, f -> core, half, f, q, band, t, u
    main = main.transpose(0, 4, 6, 1, 2, 3, 5).reshape(
        N_CORES, P, NQ_FULL * 2048)
    tail = x[:, NQ_FULL * CHN:].reshape(N_CORES, 2, 2, 2, TAIL_U, F_NODE)
    tail = tail.transpose(0, 3, 5, 1, 2, 4).reshape(N_CORES, P, 4 * TAIL_U)
    return main.astype(NPF8), tail.astype(NPF8)


def _prep_in_maps(node_features, col_features, W1, b1, W2, b2,
                  node_fc_W, col_W1, col_b1, col_W2):
    W1s = np.asarray(W1, np.float32) * WS
    W2s = np.asarray(W2, np.float32) * WS

    wl1 = np.zeros((P, 2, 64), np.float32)
    wl1[0:64, 0, 0:H] = W1s
    wl1[64:128, 0, H:2 * H] = W1s
    wl1[0:64, 1, 2 * H:3 * H] = W1s
    wl1[64:128, 1, 3 * H:4 * H] = W1s
    wl2 = np.zeros((P, P), np.float32)
    for g in range(P // H):
        wl2[H * g:H * g + H, H * g:H * g + H] = W2s

    wbx_base = np.zeros((P, WBX), np.float32)
    wbx_base[0:F_COL, 0:H] = np.asarray(col_W1, np.float32)
    wbx_base[:, H + COLN] = np.tile(np.asarray(node_fc_W, np.float32)[:, 0],
                                    P // H) / (WS * WS * np.float32(N))
    wbx_base[0:H, H + COLN + 1] = np.asarray(col_W2, np.float32)[:, 0] \
        / np.float32(C)
    wbx_base[:, H + COLN + 2] = WS * np.tile(np.asarray(b1, np.float32),
                                             P // H)
    wbx_base[:, H + COLN + 3] = WS * WS * np.tile(np.asarray(b2, np.float32),
                                                  P // H)
    wbx_base[0:H, H + COLN + 4] = np.asarray(col_b1, np.float32)

    colf = np.ascontiguousarray(col_features, np.float32).reshape(B * C, F_COL)
    xmain, xtail = _pack_x(node_features)

    base = np.zeros((P, TOT), NPF8)
    bb = base.view(np.uint8)
    bb[:, 0:128] = wl1.reshape(P, 128).astype(NPF8).view(np.uint8)
    bb[:, 128:256] = wl2.astype(NPF8).view(np.uint8)

    in_maps = []
    for c in range(N_CORES):
        arr = base.copy()
        ab = arr.view(np.uint8)
        wbx = wbx_base.copy()
        wbx[0:F_COL, H:H + COLN] = colf[c * COLN:(c + 1) * COLN].T
        ab[:, BF_OFF:BF_OFF + 2 * WBX] = \
            wbx.astype(NPBF).view(np.uint8).reshape(P, 2 * WBX)
        ab[:, X0:XT] = xmain[c].view(np.uint8)
        ab[:, XT:TOT] = xtail[c].view(np.uint8)
        in_maps.append({"xw8": arr})
    return in_maps


def kernel(node_features, col_features, edge_index, W1, b1, W2, b2,
           node_fc_W, node_fc_b, col_W1, col_b1, col_W2, col_b2,
           fc_W, fc_b, out_W, out_b):
    global LAST_EXEC_TIME_NS, LAST_RESULTS
    # edge_index provably does not affect the output (see module docstring).
    in_maps = _prep_in_maps(node_features, col_features, W1, b1, W2, b2,
                            node_fc_W, col_W1, col_b1, col_W2)
    with_bias = bool(np.any(np.asarray(b1)) or np.any(np.asarray(b2))
                     or np.any(np.asarray(col_b1)))
    nc = _get_nc(with_bias)
    res = run_bass_kernel_spmd(nc, in_maps, core_ids=list(range(N_CORES)),
                               trace=PROFILE)
    LAST_EXEC_TIME_NS = res.exec_time_ns
    LAST_RESULTS = res
    outs = res.results

    node_avg = np.zeros((B, 1), np.float32)
    col_avg = np.zeros((B, 1), np.float32)
    nfb = np.asarray(node_fc_b, np.float32)[0]
    cb2 = np.asarray(col_b2, np.float32)[0]
    for b in range(B):
        o0 = outs[2 * b]["out"].reshape(2)
        o1 = outs[2 * b + 1]["out"].reshape(2)
        node_avg[b, 0] = o0[0] + o1[0] + nfb
        col_avg[b, 0] = o0[1] + o1[1] + cb2

    combined = np.concatenate([node_avg, col_avg], axis=1)      # [B, 2]
    z = np.maximum(combined @ np.asarray(fc_W, np.float32) +
                   np.asarray(fc_b, np.float32), 0.0)
    out = z @ np.asarray(out_W, np.float32) + np.asarray(out_b, np.float32)
    return out.astype(np.float32)


# revision 15
# speedup vs baseline: 1.1728x; 1.1728x over previous
"""Trainium2 Bass kernel for nn_CriticNetwork (gnn_message_passing).

Math (verified vs reference): the reference broadcasts edge_index to
(B, 2, E) and reshapes to (2, B*E); row-major interleaving makes src and
dst elementwise equal, so every edge is a self-edge and both GCNConv
layers collapse to plain linear layers (deg*x/deg = x):

    x = relu(x @ W1); x = relu(x @ W2)        (biases are zeros)
    node_avg[b] = mean_n(x[b,n] @ node_fc_W) + node_fc_b
    col path is a tiny 2-layer MLP; final head is a [4,2] MLP.

Device strategy (per core, 25000 nodes = half a batch):
  * node path in fp8e4m3 end to end (x, W1*16, W2*16, hidden acts);
    ~2e-3 final rel err vs the 2e-2 gate.  The tiny col path stays bf16
    (quantizing it dominates the error budget).
  * L1 uses MatmulPerfMode.DoubleRow: 256-deep contraction packs FOUR
    nodes per output column.  Outputs must start at PSUM partition 0,
    so the two 4-node bands go to separate [64,512] banks; relu1 band A
    runs on ACT and band B on DVE, writing halves of one dense
    [128,512] h1r tile; L2 is one plain fp8 matmul per chunk; relu2 +
    row-sum accum alternates ACT/DVE by chunk parity.
  * ALL inputs ship in ONE fp8 DRAM tensor (weights fp8 | bf16 block as
    raw bytes via AP.bitcast | x blocks), moved by 5 column-range DMAs
    round-robined over the two HWDGE queues (Sync + Scalar) so the
    hardware descriptor generators work in parallel; x stays fully
    resident in SBUF (no pool recycling, DMA never stalls on compute).
  * the final reduction is done ON the PE: stats row-sums (bf16) dot
    node_fc_W/(256N) and col_W2/C, so the output DMA is a single [1,2]
    fp32 descriptor.  (A [128,1] output DMA costs ~128 descriptors and
    microseconds of tail latency.)
"""

import ml_dtypes
import numpy as np

import concourse.bacc as bacc
import concourse.bass as bass
import concourse.mybir as mybir
import concourse.tile as tile
from concourse.bass_utils import run_bass_kernel_spmd

P = 128
N_CORES = 8
B, N, F_NODE, H = 4, 50000, 64, 16
C, F_COL = 1000, 32
NODES = (B * N) // N_CORES                 # 25000 nodes per core
CHN = 4096                                 # nodes per PSUM chunk (8/col x 512)
NQ_FULL = NODES // CHN                     # 6 full chunks
TAIL_N = NODES - NQ_FULL * CHN             # 424
TAIL_U = TAIL_N // 8                       # 53 output columns in tail chunk
NCH = NQ_FULL + 1                          # 7 chunks total
COLN = (B * C) // N_CORES                  # 500 col-feature rows per core
WS = 16.0                                  # pow2 weight scale for fp8

F8 = mybir.dt.float8e4
NPF8 = ml_dtypes.float8_e4m3               # matches mybir.dt.np(float8e4)
BF = mybir.dt.bfloat16
NPBF = ml_dtypes.bfloat16

# Single packed input tensor xw8 [128, TOT] fp8 (per core), columns:
#   [0:128)        wl1: W1 DoubleRow blockdiag ([128,2,64] view)
#   [128:256)      wl2: blockdiag(W2 x8) flattened
#   [256:257)      zeros column (warmup operand)
#   [320:...)      bf16 block as raw bytes (bitcast view [128, WBX]):
#                  bf16 cols 0:16 col_W1 (rows 0-31), 16:516 colT
#                  (rows 0-31), 516 wn, 517 wc, 518 b1s, 519 b2s, 520 cb1
#   [X0:X0+12288)  x main blocks [24, 512]
#   [XT:XT+212)    x tail blocks [4, 53]
WBX = H + COLN + 5                          # 521 bf16 columns
BF_OFF = 320
X0 = BF_OFF + 2 * WBX                       # 1362 -> pad to 1364
X0 = (X0 + 3) & ~3                          # 1364
XT = X0 + NQ_FULL * 2048                    # 13652
TOT = XT + 4 * TAIL_U                       # 13864

PROFILE = False
CHECK_WAITS = True
LAST_EXEC_TIME_NS = None
LAST_RESULTS = None

_NC_CACHE = {}


def _build_nc(with_bias=False):
    f32 = mybir.dt.float32
    Relu = mybir.ActivationFunctionType.Relu
    DR = mybir.MatmulPerfMode.DoubleRow
    nc = bacc.Bacc("TRN2")

    xw8 = nc.dram_tensor("xw8", [P, TOT], F8, kind="ExternalInput")
    out = nc.dram_tensor("out", [1, 2], f32, kind="ExternalOutput")

    with tile.TileContext(nc) as tc:
        with (
            tc.tile_pool(name="consts", bufs=1) as consts,
            tc.tile_pool(name="xin", bufs=1) as xin,
            tc.tile_pool(name="work", bufs=1) as work,
            tc.tile_pool(name="psum", bufs=1, space="PSUM") as psum,
        ):
            xf = xin.tile([P, TOT], F8)
            stats = consts.tile([P, NCH + 1], f32)
            node_sb = consts.tile([P, 1], BF)
            col_sb = consts.tile([H, 1], f32)
            col_sbb = consts.tile([H, 1], BF)
            outsb = consts.tile([1, 2], f32)
            colscr = consts.tile([H, COLN], BF)
            h1r = [work.tile([P, 512], F8, tag=f"h1r{k}", name=f"h1r{k}")
                   for k in range(3)]
            scr = [work.tile([P, 512], F8, tag=f"scr{k}", name=f"scr{k}")
                   for k in range(2)]

            # DoubleRow matmul outputs must start at PSUM partition 0, so
            # the two 4-node bands land in separate half-used banks.
            ps1a = [psum.tile([64, 512], f32, tag=f"ps1a{k}", name=f"ps1a{k}")
                    for k in range(2)]
            ps1b = [psum.tile([64, 512], f32, tag=f"ps1b{k}", name=f"ps1b{k}")
                    for k in range(2)]
            ps2 = [psum.tile([P, 512], f32, tag=f"ps2_{k}", name=f"ps2_{k}")
                   for k in range(3)]
            pscm = psum.tile([H, 512], f32, tag="pscm")
            psc = pscm[:, 0:COLN]
            psd = pscm[0:1, COLN:COLN + 1]
            psdot = pscm[0:1, COLN + 1:COLN + 3]

            wl1 = xf[:, 0:128].rearrange("p (t m) -> p t m", t=2)
            wl2 = xf[:, 128:256]
            zc = xf[:, 256:257]
            wbx = xf[:, BF_OFF:BF_OFF + 2 * WBX].bitcast(BF)
            cw1 = wbx[0:F_COL, 0:H]
            colT = wbx[0:F_COL, H:H + COLN]
            wn = wbx[:, H + COLN:H + COLN + 1]
            wc = wbx[0:H, H + COLN + 1:H + COLN + 2]
            b1s = wbx[:, H + COLN + 2:H + COLN + 3]
            b2s = wbx[:, H + COLN + 3:H + COLN + 4]
            cb1 = wbx[0:H, H + COLN + 4:H + COLN + 5]

            # stats zeroed on GpSimd: no data deps, runs during DMA window.
            nc.gpsimd.memset(stats[:], 0.0)

            # 5 column-range DMA pieces round-robined over the two HWDGE
            # queues (parallel descriptor generation): chunk 0 first.
            nc.sync.dma_start(xf[:, X0:X0 + 2048], xw8[:, X0:X0 + 2048])
            nc.scalar.dma_start(xf[:, 0:X0], xw8[:, 0:X0])
            nc.sync.dma_start(xf[:, X0 + 2048:X0 + 6144],
                              xw8[:, X0 + 2048:X0 + 6144])
            nc.scalar.dma_start(xf[:, X0 + 6144:X0 + 10240],
                                xw8[:, X0 + 6144:X0 + 10240])
            nc.sync.dma_start(xf[:, X0 + 10240:TOT], xw8[:, X0 + 10240:TOT])

            # PE p-state warming: the Tensor engine only reaches its top
            # clock after ~3us of CONTINUOUS execution.  While the first
            # chunk's DMA is in flight (~4.5us), stream dependency-free
            # dummy matmuls over uninitialized SBUF junk (results may be
            # NaN; they land in pscm[0:1, 0:500], which the col-path
            # matmul later resets with start=True).  Real matmuls then
            # run at the ramped clock instead of the mid p-state.
            jw = scr[0][:, 0:1]
            jr = scr[1][:, 0:500]
            for k in range(9):
                nc.tensor.matmul(pscm[0:1, 0:500], jw, jr,
                                 start=True, stop=True, skip_group_check=True)
            for k in range(3):
                nc.tensor.matmul(pscm[0:1, 0:128], jw, jr[:, 0:128],
                                 start=True, stop=True, skip_group_check=True)

            # Warmup matmul: subsumes the consts-piece DMA wait into PE
            # program order with a single semaphore wait.
            nc.tensor.matmul(psd, zc, zc, start=True, stop=True)
            nc.vector.tensor_copy(stats[0:1, NCH:NCH + 1], psd)

            for s in range(NCH):
                u = 512 if s < NQ_FULL else TAIL_U
                base = X0 + 2048 * s
                xa = xf[:, base:base + 2 * u].rearrange("p (t u) -> p t u", t=2)
                xb = xf[:, base + 2 * u:base + 4 * u].rearrange(
                    "p (t u) -> p t u", t=2)
                pa = ps1a[s % 2]
                pb = ps1b[s % 2]
                nc.tensor.matmul(pa[:, :u], wl1, xa, start=True, stop=True,
                                 perf_mode=DR, tile_position=(0, 0))
                nc.tensor.matmul(pb[:, :u], wl1, xb, start=True, stop=True,
                                 perf_mode=DR, tile_position=(0, 0))
                h = h1r[s % 3]
                # relu1 split across the two PSUM-capable engines: band A
                # on ACT, band B on DVE; both write halves of one h1r tile.
                nc.scalar.activation(h[0:64, :u], pa[:, :u], Relu,
                                     bias=b1s[0:64] if with_bias else 0.0)
                if with_bias:
                    # b1s is 16-periodic, so rows 0:64 serve band B as well
                    nc.vector.tensor_scalar(
                        h[64:128, :u], pb[:, :u], b1s[0:64], 0.0,
                        mybir.AluOpType.add, mybir.AluOpType.max)
                else:
                    nc.vector.tensor_scalar_max(h[64:128, :u], pb[:, :u], 0.0)
                p2 = ps2[s % 3]
                nc.tensor.matmul(p2[:, :u], wl2, h[:, :u], start=True, stop=True)
                # relu2 + row-sum accumulate: alternate engines by parity
                # to balance ACT vs DVE load.
                if with_bias:
                    if s % 2 == 0:
                        nc.vector.tensor_scalar(
                            scr[s % 2][:, :u], p2[:, :u], b2s, 0.0,
                            mybir.AluOpType.add, mybir.AluOpType.max,
                            accum_out=stats[:, s:s + 1])
                    else:
                        nc.scalar.activation(
                            scr[s % 2][:, :u], p2[:, :u], Relu, bias=b2s,
                            accum_out=stats[:, s:s + 1])
                else:
                    if s % 2 == 0:
                        nc.vector.tensor_scalar(
                            scr[s % 2][:, :u], p2[:, :u], 0.0, 0.0,
                            mybir.AluOpType.max, mybir.AluOpType.add,
                            accum_out=stats[:, s:s + 1])
                    else:
                        nc.scalar.activation(
                            scr[s % 2][:, :u], p2[:, :u], Relu,
                            accum_out=stats[:, s:s + 1])
                if s == 2:
                    # col path (bf16, tiny): fits in PE/ACT slack mid-stream
                    nc.tensor.matmul(psc[:, :], cw1, colT, start=True,
                                     stop=True)
                    nc.scalar.activation(colscr[:], psc[:], Relu,
                                         bias=cb1 if with_bias else 0.0,
                                         accum_out=col_sb[:])
                    nc.vector.tensor_copy(col_sbb[:], col_sb[:])

            # Final reductions on-chip: bf16 row sums, then PE dot products
            # so the output is a single tiny [1,2] DMA (one descriptor).
            with nc.allow_low_precision(
                    reason="bf16 row-sums feed a bf16 PE dot; 8 fp32 "
                           "terms/partition, ~0.4% quantization on a "
                           "2e-2 budget"):
                nc.vector.tensor_reduce(node_sb[:], stats[:],
                                        axis=mybir.AxisListType.X,
                                        op=mybir.AluOpType.add)
            nc.tensor.matmul(psdot[0:1, 0:1], node_sb[:], wn,
                             start=True, stop=True)
            nc.tensor.matmul(psdot[0:1, 1:2], col_sbb[:], wc,
                             start=True, stop=True)
            nc.scalar.copy(outsb[:], psdot[:])
            nc.sync.dma_start(out[:], outsb[:])

    nc.finalize()

    if CHECK_WAITS:
        for blk in nc.m.functions[0].blocks:
            for inst in blk.instructions:
                si = inst.sync_info
                nwait = len(si.on_wait) if si and si.on_wait else 0
                limit = 2 if type(inst).__name__ in (
                    "InstEventSemaphore", "InstDrain", "InstDMACopy") else 1
                assert nwait <= limit, (
                    inst.name, type(inst).__name__,
                    [w.ant_name for w in si.on_wait])
    return nc


def _get_nc(with_bias=False):
    key = ("nc", with_bias)
    if key not in _NC_CACHE:
        _NC_CACHE[key] = _build_nc(with_bias)
    return _NC_CACHE[key]


def _pack_x(node_features):
    """[B,N,64] -> per-core [128, 12288] + [128, 212] fp8 x payloads.

    Node n (within a core) lives at chunk q = n // 4096, slot
    s = (n % 4096) // 512 = band*4 + t*2 + half, column u = n % 512:
    columns q*2048 + band*1024 + t*512 + u, partitions half*64 + f.
    """
    x = np.ascontiguousarray(node_features, np.float32).reshape(
        N_CORES, NODES, F_NODE)
    main = x[:, :NQ_FULL * CHN].reshape(N_CORES, NQ_FULL, 2, 2, 2, 512, F_NODE)
    #                  core, q, band, t, half, u, f -> core, half, f, q, band, t, u
    main = main.transpose(0, 4, 6, 1, 2, 3, 5).reshape(
        N_CORES, P, NQ_FULL * 2048)
    tail = x[:, NQ_FULL * CHN:].reshape(N_CORES, 2, 2, 2, TAIL_U, F_NODE)
    tail = tail.transpose(0, 3, 5, 1, 2, 4).reshape(N_CORES, P, 4 * TAIL_U)
    return main.astype(NPF8), tail.astype(NPF8)


def _prep_in_maps(node_features, col_features, W1, b1, W2, b2,
                  node_fc_W, col_W1, col_b1, col_W2):
    W1s = np.asarray(W1, np.float32) * WS
    W2s = np.asarray(W2, np.float32) * WS

    wl1 = np.zeros((P, 2, 64), np.float32)
    wl1[0:64, 0, 0:H] = W1s
    wl1[64:128, 0, H:2 * H] = W1s
    wl1[0:64, 1, 2 * H:3 * H] = W1s
    wl1[64:128, 1, 3 * H:4 * H] = W1s
    wl2 = np.zeros((P, P), np.float32)
    for g in range(P // H):
        wl2[H * g:H * g + H, H * g:H * g + H] = W2s

    wbx_base = np.zeros((P, WBX), np.float32)
    wbx_base[0:F_COL, 0:H] = np.asarray(col_W1, np.float32)
    wbx_base[:, H + COLN] = np.tile(np.asarray(node_fc_W, np.float32)[:, 0],
                                    P // H) / (WS * WS * np.float32(N))
    wbx_base[0:H, H + COLN + 1] = np.asarray(col_W2, np.float32)[:, 0] \
        / np.float32(C)
    wbx_base[:, H + COLN + 2] = WS * np.tile(np.asarray(b1, np.float32),
                                             P // H)
    wbx_base[:, H + COLN + 3] = WS * WS * np.tile(np.asarray(b2, np.float32),
                                                  P // H)
    wbx_base[0:H, H + COLN + 4] = np.asarray(col_b1, np.float32)

    colf = np.ascontiguousarray(col_features, np.float32).reshape(B * C, F_COL)
    xmain, xtail = _pack_x(node_features)

    base = np.zeros((P, TOT), NPF8)
    bb = base.view(np.uint8)
    bb[:, 0:128] = wl1.reshape(P, 128).astype(NPF8).view(np.uint8)
    bb[:, 128:256] = wl2.astype(NPF8).view(np.uint8)

    in_maps = []
    for c in range(N_CORES):
        arr = base.copy()
        ab = arr.view(np.uint8)
        wbx = wbx_base.copy()
        wbx[0:F_COL, H:H + COLN] = colf[c * COLN:(c + 1) * COLN].T
        ab[:, BF_OFF:BF_OFF + 2 * WBX] = \
            wbx.astype(NPBF).view(np.uint8).reshape(P, 2 * WBX)
        ab[:, X0:XT] = xmain[c].view(np.uint8)
        ab[:, XT:TOT] = xtail[c].view(np.uint8)
        in_maps.append({"xw8": arr})
    return in_maps


def kernel(node_features, col_features, edge_index, W1, b1, W2, b2,
           node_fc_W, node_fc_b, col_W1, col_b1, col_W2, col_b2,
           fc_W, fc_b, out_W, out_b):
    global LAST_EXEC_TIME_NS, LAST_RESULTS
    # edge_index provably does not affect the output (see module docstring).
    in_maps = _prep_in_maps(node_features, col_features, W1, b1, W2, b2,
                            node_fc_W, col_W1, col_b1, col_W2)
    with_bias = bool(np.any(np.asarray(b1)) or np.any(np.asarray(b2))
                     or np.any(np.asarray(col_b1)))
    nc = _get_nc(with_bias)
    res = run_bass_kernel_spmd(nc, in_maps, core_ids=list(range(N_CORES)),
                               trace=PROFILE)
    LAST_EXEC_TIME_NS = res.exec_time_ns
    LAST_RESULTS = res
    outs = res.results

    node_avg = np.zeros((B, 1), np.float32)
    col_avg = np.zeros((B, 1), np.float32)
    nfb = np.asarray(node_fc_b, np.float32)[0]
    cb2 = np.asarray(col_b2, np.float32)[0]
    for b in range(B):
        o0 = outs[2 * b]["out"].reshape(2)
        o1 = outs[2 * b + 1]["out"].reshape(2)
        node_avg[b, 0] = o0[0] + o1[0] + nfb
        col_avg[b, 0] = o0[1] + o1[1] + cb2

    combined = np.concatenate([node_avg, col_avg], axis=1)      # [B, 2]
    z = np.maximum(combined @ np.asarray(fc_W, np.float32) +
                   np.asarray(fc_b, np.float32), 0.0)
    out = z @ np.asarray(out_W, np.float32) + np.asarray(out_b, np.float32)
    return out.astype(np.float32)


# revision 16
# speedup vs baseline: 1.2371x; 1.0549x over previous
"""Trainium2 Bass kernel for nn_CriticNetwork (gnn_message_passing).

Math (verified vs reference): the reference broadcasts edge_index to
(B, 2, E) and reshapes to (2, B*E); row-major interleaving makes src and
dst elementwise equal, so every edge is a self-edge and both GCNConv
layers collapse to plain linear layers (deg*x/deg = x):

    x = relu(x @ W1); x = relu(x @ W2)        (biases are zeros)
    node_avg[b] = mean_n(x[b,n] @ node_fc_W) + node_fc_b
    col path is a tiny 2-layer MLP; final head is a [4,2] MLP.

Device strategy (per core, 25000 nodes = half a batch):
  * node path in fp8e4m3 end to end (x, W1*16, W2*16, hidden acts);
    ~2e-3 final rel err vs the 2e-2 gate.  The tiny col path stays bf16
    (quantizing it dominates the error budget).
  * L1 uses MatmulPerfMode.DoubleRow: 256-deep contraction packs FOUR
    nodes per output column.  Outputs must start at PSUM partition 0,
    so the two 4-node bands go to separate [64,512] banks; relu1 band A
    runs on ACT and band B on DVE, writing halves of one dense
    [128,512] h1r tile; L2 is one plain fp8 matmul per chunk; relu2 +
    row-sum accum alternates ACT/DVE by chunk parity.
  * ALL inputs ship in ONE fp8 DRAM tensor (weights fp8 | bf16 block as
    raw bytes via AP.bitcast | x blocks), moved by 5 column-range DMAs
    round-robined over the two HWDGE queues (Sync + Scalar) so the
    hardware descriptor generators work in parallel; x stays fully
    resident in SBUF (no pool recycling, DMA never stalls on compute).
  * the final reduction is done ON the PE: stats row-sums (bf16) dot
    node_fc_W/(256N) and col_W2/C, so the output DMA is a single [1,2]
    fp32 descriptor.  (A [128,1] output DMA costs ~128 descriptors and
    microseconds of tail latency.)
"""

import ml_dtypes
import numpy as np

import concourse.bacc as bacc
import concourse.bass as bass
import concourse.mybir as mybir
import concourse.tile as tile
from concourse.bass_utils import run_bass_kernel_spmd

P = 128
N_CORES = 8
B, N, F_NODE, H = 4, 50000, 64, 16
C, F_COL = 1000, 32
NODES = (B * N) // N_CORES                 # 25000 nodes per core
CHN = 4096                                 # nodes per PSUM chunk (8/col x 512)
NQ_FULL = NODES // CHN                     # 6 full chunks
TAIL_N = NODES - NQ_FULL * CHN             # 424
TAIL_U = TAIL_N // 8                       # 53 output columns in tail chunk
NCH = NQ_FULL + 1                          # 7 chunks total
COLN = (B * C) // N_CORES                  # 500 col-feature rows per core
WS = 16.0                                  # pow2 weight scale for fp8

F8 = mybir.dt.float8e4
NPF8 = ml_dtypes.float8_e4m3               # matches mybir.dt.np(float8e4)
BF = mybir.dt.bfloat16
NPBF = ml_dtypes.bfloat16

# Single packed input tensor xw8 [128, TOT] fp8 (per core), columns:
#   [0:128)        wl1: W1 DoubleRow blockdiag ([128,2,64] view)
#   [128:256)      wl2: blockdiag(W2 x8) flattened
#   [256:257)      zeros column (warmup operand)
#   [320:...)      bf16 block as raw bytes (bitcast view [128, WBX]):
#                  bf16 cols 0:16 col_W1 (rows 0-31), 16:516 colT
#                  (rows 0-31), 516 wn, 517 wc, 518 b1s, 519 b2s, 520 cb1
#   [X0:X0+12288)  x main blocks [24, 512]
#   [XT:XT+212)    x tail blocks [4, 53]
WBX = H + COLN + 5                          # 521 bf16 columns
BF_OFF = 320
X0 = BF_OFF + 2 * WBX                       # 1362 -> pad to 1364
X0 = (X0 + 3) & ~3                          # 1364
XT = X0 + NQ_FULL * 2048                    # 13652
TOT = XT + 4 * TAIL_U                       # 13864

PROFILE = False
CHECK_WAITS = True
LAST_EXEC_TIME_NS = None
LAST_RESULTS = None

_NC_CACHE = {}


def _build_nc(with_bias=False):
    f32 = mybir.dt.float32
    Relu = mybir.ActivationFunctionType.Relu
    DR = mybir.MatmulPerfMode.DoubleRow
    nc = bacc.Bacc("TRN2")

    xw8 = nc.dram_tensor("xw8", [P, TOT], F8, kind="ExternalInput")
    out = nc.dram_tensor("out", [1, 2], f32, kind="ExternalOutput")

    with tile.TileContext(nc) as tc:
        with (
            tc.tile_pool(name="consts", bufs=1) as consts,
            tc.tile_pool(name="xin", bufs=1) as xin,
            tc.tile_pool(name="work", bufs=1) as work,
            tc.tile_pool(name="psum", bufs=1, space="PSUM") as psum,
        ):
            xf = xin.tile([P, TOT], F8)
            stats = consts.tile([P, NCH + 1], f32)
            node_sb = consts.tile([P, 1], BF)
            col_sb = consts.tile([H, 1], f32)
            col_sbb = consts.tile([H, 1], BF)
            outsb = consts.tile([1, 2], f32)
            colscr = consts.tile([H, COLN], BF)
            h1r = [work.tile([P, 512], F8, tag=f"h1r{k}", name=f"h1r{k}")
                   for k in range(3)]
            scr = [work.tile([P, 512], F8, tag=f"scr{k}", name=f"scr{k}")
                   for k in range(2)]

            # DoubleRow matmul outputs must start at PSUM partition 0, so
            # the two 4-node bands land in separate half-used banks.
            ps1a = [psum.tile([64, 512], f32, tag=f"ps1a{k}", name=f"ps1a{k}")
                    for k in range(2)]
            ps1b = [psum.tile([64, 512], f32, tag=f"ps1b{k}", name=f"ps1b{k}")
                    for k in range(2)]
            ps2 = [psum.tile([P, 512], f32, tag=f"ps2_{k}", name=f"ps2_{k}")
                   for k in range(3)]
            pscm = psum.tile([H, 512], f32, tag="pscm")
            psc = pscm[:, 0:COLN]
            psd = pscm[0:1, COLN:COLN + 1]
            psdot = pscm[0:1, COLN + 1:COLN + 3]

            wl1 = xf[:, 0:128].rearrange("p (t m) -> p t m", t=2)
            wl2 = xf[:, 128:256]
            zc = xf[:, 256:257]
            wbx = xf[:, BF_OFF:BF_OFF + 2 * WBX].bitcast(BF)
            cw1 = wbx[0:F_COL, 0:H]
            colT = wbx[0:F_COL, H:H + COLN]
            wn = wbx[:, H + COLN:H + COLN + 1]
            wc = wbx[0:H, H + COLN + 1:H + COLN + 2]
            b1s = wbx[:, H + COLN + 2:H + COLN + 3]
            b2s = wbx[:, H + COLN + 3:H + COLN + 4]
            cb1 = wbx[0:H, H + COLN + 4:H + COLN + 5]

            # stats zeroed on GpSimd: no data deps, runs during DMA window.
            nc.gpsimd.memset(stats[:], 0.0)

            # 5 column-range DMA pieces round-robined over the two HWDGE
            # queues (parallel descriptor generation): chunk 0 first.
            nc.sync.dma_start(xf[:, X0:X0 + 2048], xw8[:, X0:X0 + 2048])
            nc.scalar.dma_start(xf[:, 0:X0], xw8[:, 0:X0])
            nc.sync.dma_start(xf[:, X0 + 2048:X0 + 6144],
                              xw8[:, X0 + 2048:X0 + 6144])
            nc.scalar.dma_start(xf[:, X0 + 6144:X0 + 10240],
                                xw8[:, X0 + 6144:X0 + 10240])
            nc.sync.dma_start(xf[:, X0 + 10240:TOT], xw8[:, X0 + 10240:TOT])

            # Warmup matmul: subsumes the consts-piece DMA wait into PE
            # program order with a single semaphore wait.
            nc.tensor.matmul(psd, zc, zc, start=True, stop=True)
            nc.vector.tensor_copy(stats[0:1, NCH:NCH + 1], psd)

            for s in range(NCH):
                u = 512 if s < NQ_FULL else TAIL_U
                base = X0 + 2048 * s
                xa = xf[:, base:base + 2 * u].rearrange("p (t u) -> p t u", t=2)
                xb = xf[:, base + 2 * u:base + 4 * u].rearrange(
                    "p (t u) -> p t u", t=2)
                pa = ps1a[s % 2]
                pb = ps1b[s % 2]
                nc.tensor.matmul(pa[:, :u], wl1, xa, start=True, stop=True,
                                 perf_mode=DR, tile_position=(0, 0))
                nc.tensor.matmul(pb[:, :u], wl1, xb, start=True, stop=True,
                                 perf_mode=DR, tile_position=(0, 0))
                h = h1r[s % 3]
                # relu1 split across the two PSUM-capable engines: band A
                # on ACT, band B on DVE; both write halves of one h1r tile.
                nc.scalar.activation(h[0:64, :u], pa[:, :u], Relu,
                                     bias=b1s[0:64] if with_bias else 0.0)
                if with_bias:
                    # b1s is 16-periodic, so rows 0:64 serve band B as well
                    nc.vector.tensor_scalar(
                        h[64:128, :u], pb[:, :u], b1s[0:64], 0.0,
                        mybir.AluOpType.add, mybir.AluOpType.max)
                else:
                    nc.vector.tensor_scalar_max(h[64:128, :u], pb[:, :u], 0.0)
                p2 = ps2[s % 3]
                nc.tensor.matmul(p2[:, :u], wl2, h[:, :u], start=True, stop=True)
                # relu2 + row-sum accumulate: alternate engines by parity
                # to balance ACT vs DVE load.
                if with_bias:
                    if s % 2 == 0:
                        nc.vector.tensor_scalar(
                            scr[s % 2][:, :u], p2[:, :u], b2s, 0.0,
                            mybir.AluOpType.add, mybir.AluOpType.max,
                            accum_out=stats[:, s:s + 1])
                    else:
                        nc.scalar.activation(
                            scr[s % 2][:, :u], p2[:, :u], Relu, bias=b2s,
                            accum_out=stats[:, s:s + 1])
                else:
                    if s % 2 == 0:
                        nc.vector.tensor_scalar(
                            scr[s % 2][:, :u], p2[:, :u], 0.0, 0.0,
                            mybir.AluOpType.max, mybir.AluOpType.add,
                            accum_out=stats[:, s:s + 1])
                    else:
                        nc.scalar.activation(
                            scr[s % 2][:, :u], p2[:, :u], Relu,
                            accum_out=stats[:, s:s + 1])
                if s == 2:
                    # col path (bf16, tiny): fits in PE/ACT slack mid-stream
                    nc.tensor.matmul(psc[:, :], cw1, colT, start=True,
                                     stop=True)
                    nc.scalar.activation(colscr[:], psc[:], Relu,
                                         bias=cb1 if with_bias else 0.0,
                                         accum_out=col_sb[:])
                    nc.vector.tensor_copy(col_sbb[:], col_sb[:])

            # Final reductions on-chip: bf16 row sums, then PE dot products
            # so the output is a single tiny [1,2] DMA (one descriptor).
            with nc.allow_low_precision(
                    reason="bf16 row-sums feed a bf16 PE dot; 8 fp32 "
                           "terms/partition, ~0.4% quantization on a "
                           "2e-2 budget"):
                nc.vector.tensor_reduce(node_sb[:], stats[:],
                                        axis=mybir.AxisListType.X,
                                        op=mybir.AluOpType.add)
            nc.tensor.matmul(psdot[0:1, 0:1], node_sb[:], wn,
                             start=True, stop=True)
            nc.tensor.matmul(psdot[0:1, 1:2], col_sbb[:], wc,
                             start=True, stop=True)
            nc.scalar.copy(outsb[:], psdot[:])
            nc.sync.dma_start(out[:], outsb[:])

    nc.finalize()

    if CHECK_WAITS:
        for blk in nc.m.functions[0].blocks:
            for inst in blk.instructions:
                si = inst.sync_info
                nwait = len(si.on_wait) if si and si.on_wait else 0
                limit = 2 if type(inst).__name__ in (
                    "InstEventSemaphore", "InstDrain", "InstDMACopy") else 1
                assert nwait <= limit, (
                    inst.name, type(inst).__name__,
                    [w.ant_name for w in si.on_wait])
    return nc


def _get_nc(with_bias=False):
    key = ("nc", with_bias)
    if key not in _NC_CACHE:
        _NC_CACHE[key] = _build_nc(with_bias)
    return _NC_CACHE[key]


def _pack_x(node_features):
    """[B,N,64] -> per-core [128, 12288] + [128, 212] fp8 x payloads.

    Node n (within a core) lives at chunk q = n // 4096, slot
    s = (n % 4096) // 512 = band*4 + t*2 + half, column u = n % 512:
    columns q*2048 + band*1024 + t*512 + u, partitions half*64 + f.
    """
    x = np.ascontiguousarray(node_features, np.float32).reshape(
        N_CORES, NODES, F_NODE)
    main = x[:, :NQ_FULL * CHN].reshape(N_CORES, NQ_FULL, 2, 2, 2, 512, F_NODE)
    #                  core, q, band, t, half, u, f -> core, half, f, q, band, t, u
    main = main.transpose(0, 4, 6, 1, 2, 3, 5).reshape(
        N_CORES, P, NQ_FULL * 2048)
    tail = x[:, NQ_FULL * CHN:].reshape(N_CORES, 2, 2, 2, TAIL_U, F_NODE)
    tail = tail.transpose(0, 3, 5, 1, 2, 4).reshape(N_CORES, P, 4 * TAIL_U)
    return main.astype(NPF8), tail.astype(NPF8)


def _prep_in_maps(node_features, col_features, W1, b1, W2, b2,
                  node_fc_W, col_W1, col_b1, col_W2):
    W1s = np.asarray(W1, np.float32) * WS
    W2s = np.asarray(W2, np.float32) * WS

    wl1 = np.zeros((P, 2, 64), np.float32)
    wl1[0:64, 0, 0:H] = W1s
    wl1[64:128, 0, H:2 * H] = W1s
    wl1[0:64, 1, 2 * H:3 * H] = W1s
    wl1[64:128, 1, 3 * H:4 * H] = W1s
    wl2 = np.zeros((P, P), np.float32)
    for g in range(P // H):
        wl2[H * g:H * g + H, H * g:H * g + H] = W2s

    wbx_base = np.zeros((P, WBX), np.float32)
    wbx_base[0:F_COL, 0:H] = np.asarray(col_W1, np.float32)
    wbx_base[:, H + COLN] = np.tile(np.asarray(node_fc_W, np.float32)[:, 0],
                                    P // H) / (WS * WS * np.float32(N))
    wbx_base[0:H, H + COLN + 1] = np.asarray(col_W2, np.float32)[:, 0] \
        / np.float32(C)
    wbx_base[:, H + COLN + 2] = WS * np.tile(np.asarray(b1, np.float32),
                                             P // H)
    wbx_base[:, H + COLN + 3] = WS * WS * np.tile(np.asarray(b2, np.float32),
                                                  P // H)
    wbx_base[0:H, H + COLN + 4] = np.asarray(col_b1, np.float32)

    colf = np.ascontiguousarray(col_features, np.float32).reshape(B * C, F_COL)
    xmain, xtail = _pack_x(node_features)

    base = np.zeros((P, TOT), NPF8)
    bb = base.view(np.uint8)
    bb[:, 0:128] = wl1.reshape(P, 128).astype(NPF8).view(np.uint8)
    bb[:, 128:256] = wl2.astype(NPF8).view(np.uint8)

    in_maps = []
    for c in range(N_CORES):
        arr = base.copy()
        ab = arr.view(np.uint8)
        wbx = wbx_base.copy()
        wbx[0:F_COL, H:H + COLN] = colf[c * COLN:(c + 1) * COLN].T
        ab[:, BF_OFF:BF_OFF + 2 * WBX] = \
            wbx.astype(NPBF).view(np.uint8).reshape(P, 2 * WBX)
        ab[:, X0:XT] = xmain[c].view(np.uint8)
        ab[:, XT:TOT] = xtail[c].view(np.uint8)
        in_maps.append({"xw8": arr})
    return in_maps


def kernel(node_features, col_features, edge_index, W1, b1, W2, b2,
           node_fc_W, node_fc_b, col_W1, col_b1, col_W2, col_b2,
           fc_W, fc_b, out_W, out_b):
    global LAST_EXEC_TIME_NS, LAST_RESULTS
    # edge_index provably does not affect the output (see module docstring).
    in_maps = _prep_in_maps(node_features, col_features, W1, b1, W2, b2,
                            node_fc_W, col_W1, col_b1, col_W2)
    with_bias = bool(np.any(np.asarray(b1)) or np.any(np.asarray(b2))
                     or np.any(np.asarray(col_b1)))
    nc = _get_nc(with_bias)
    res = run_bass_kernel_spmd(nc, in_maps, core_ids=list(range(N_CORES)),
                               trace=PROFILE)
    LAST_EXEC_TIME_NS = res.exec_time_ns
    LAST_RESULTS = res
    outs = res.results

    node_avg = np.zeros((B, 1), np.float32)
    col_avg = np.zeros((B, 1), np.float32)
    nfb = np.asarray(node_fc_b, np.float32)[0]
    cb2 = np.asarray(col_b2, np.float32)[0]
    for b in range(B):
        o0 = outs[2 * b]["out"].reshape(2)
        o1 = outs[2 * b + 1]["out"].reshape(2)
        node_avg[b, 0] = o0[0] + o1[0] + nfb
        col_avg[b, 0] = o0[1] + o1[1] + cb2

    combined = np.concatenate([node_avg, col_avg], axis=1)      # [B, 2]
    z = np.maximum(combined @ np.asarray(fc_W, np.float32) +
                   np.asarray(fc_b, np.float32), 0.0)
    out = z @ np.asarray(out_W, np.float32) + np.asarray(out_b, np.float32)
    return out.astype(np.float32)
